# revision 1
# baseline (speedup 1.0000x reference)
"""MoE grouped linear (gmm) kernel for 8 Trainium2 NeuronCores.

Strategy (expert parallel, mirrors the shard_map-over-gmm_sharded source):
  - Tokens arrive pre-sorted by expert; group_sizes[e] tokens belong to
    expert e. Core e gets weight[e] plus expert e's token slice, padded to
    MAXG rows so all 8 cores run one SPMD program. The "all-to-all" routing
    is host-side slicing, since kernel() sees the full inputs.
  - Per core we compute y_e^T = W_e^T @ X_e^T (out^T orientation): the
    weight tiles are the PE's stationary operand in natural [K, O] layout
    and X^T (prepared host-side) streams as the moving operand.
  - fp32 inputs are DMA'd untouched into resident SBUF tiles; the PE reads
    the high half of each fp32 word as bf16 through a bitcast + stride-2
    access pattern (truncation toward zero). The mean truncation shrink is
    measured host-side and compensated via the ScalarE evacuation scale;
    the per-partition bias is fused into the same instruction. PSUM
    accumulates in fp32.
Host then unpads/concatenates per-expert outputs back to [T, Out] fp32.
"""

import numpy as np

import concourse.bass as bass
from concourse import bacc
import concourse.mybir as mybir
import concourse.tile as tile
from concourse.bass_utils import run_bass_kernel_spmd

N_CORES = 8
P = 128

_BUILD_CACHE: dict = {}


def _t_chunks(maxg: int) -> list[tuple[int, int]]:
    """Split the token free-dim into PSUM-bank-sized (<=512) chunks."""
    n = (maxg + 511) // 512
    base = ((maxg // n + P - 1) // P) * P
    chunks = []
    off = 0
    while off < maxg:
        sz = min(base, maxg - off)
        chunks.append((off, sz))
        off += sz
    return chunks


def _build_program(maxg: int, n_in: int, n_out: int):
    kb = n_in // P   # contraction blocks
    ob = n_out // P  # output-row blocks
    f32 = mybir.dt.float32
    bf16 = mybir.dt.bfloat16

    nc = bacc.Bacc(
        "TRN2", target_bir_lowering=False, debug=False, num_devices=N_CORES
    )
    xt = nc.dram_tensor("xt", [n_in, maxg], f32, kind="ExternalInput")
    # W pre-tiled host-side: [ob, P(partition of k-block), kb, P(o)] so each
    # o-slab DMA is fully contiguous per partition (8 KiB segments).
    w = nc.dram_tensor("w", [ob, P, kb, P], f32, kind="ExternalInput")
    bias = nc.dram_tensor("bias", [P, ob], f32, kind="ExternalInput")
    sc = nc.dram_tensor("sc", [P, 2], f32, kind="ExternalInput")
    yt = nc.dram_tensor("yt", [n_out, maxg], f32, kind="ExternalOutput")

    chunks = _t_chunks(maxg)

    # o-blocks processed concurrently in group 0; 8 PSUM banks available.
    GRP = max(1, min(4, 8 // len(chunks), ob))
    # k-slabs per X quarter-tile.
    XQ = next(q for q in (4, 2, 1) if kb % q == 0)

    with tile.TileContext(nc) as tc:
        with (
            tc.tile_pool(name="const", bufs=1) as constp,
            tc.tile_pool(name="xtsb", bufs=1) as xtp,
            tc.tile_pool(name="wsb", bufs=2 * GRP) as wp,
            tc.tile_pool(name="outsb", bufs=2 * GRP) as outp,
            tc.tile_pool(name="wbsb", bufs=4) as wbp,
            tc.tile_pool(name="psum", bufs=1, space="PSUM") as psump,
        ):
            bias_sb = constp.tile([P, ob], f32)
            nc.scalar.dma_start(bias_sb[:], bias[:])
            sc_sb = constp.tile([P, 2], f32)
            nc.scalar.dma_start(sc_sb[:], sc[:])

            def load_w(o):
                w_o = wp.tile([P, kb, P], f32, tag="wo", name=f"w{o}")
                nc.sync.dma_start(w_o[:], w[o])
                return w_o

            def load_xq(q):
                xq = xtp.tile([P, XQ, maxg], f32, tag=f"xq{q}", name=f"xq{q}")
                nc.sync.dma_start(
                    xq[:],
                    xt[q * XQ * P : (q + 1) * XQ * P, :].rearrange(
                        "(k p) t -> p k t", p=P
                    ),
                )
                return xq[:].bitcast(bf16).rearrange(
                    "p k (t two) -> p k t two", two=2
                )

            # Interleave the prologue DMAs: the single HW ring delivers in
            # order, and group 0 needs w0..w3 plus all of X before its end.
            nq = kb // XQ
            prologue = []
            wi = xi = 0
            while wi < GRP or xi < nq:
                if wi < GRP:
                    prologue.append(("w", wi)); wi += 1
                if xi < nq:
                    prologue.append(("x", xi)); xi += 1
            w_pref = {}
            xqs = [None] * nq
            for kind, i in prologue:
                if kind == "w":
                    w_pref[i] = load_w(i)
                else:
                    xqs[i] = load_xq(i)

            # HAM warmup: dummy matmuls with no data deps run while the
            # prologue DMAs stream, so the PE clock is at 2.4 GHz (and the
            # activity window warm) when the first real matmul issues.
            warm = constp.tile([P, 512], bf16)
            nc.gpsimd.memset(warm[:], 0)
            ps_warm = psump.tile([P, 512], f32, tag="ps0_0", name="warmps")
            for i in range(24):
                nc.tensor.matmul(
                    ps_warm[:],
                    warm[:, :P],
                    warm[:],
                    start=(i == 0),
                    stop=(i == 23),
                )

            def evac(ps, o, t0, tsz, engine, sci=0):
                """PSUM -> SBUF with fused scale + per-o bias, then store."""
                ot = outp.tile([P, tsz], f32, tag="ot", name=f"ot{o}_{t0}")
                if engine == 0:
                    nc.scalar.activation(
                        ot[:],
                        ps[:],
                        mybir.ActivationFunctionType.Identity,
                        bias=bias_sb[:, o : o + 1],
                        scale=sc_sb[:, sci : sci + 1],
                    )
                else:
                    nc.vector.tensor_scalar(
                        ot[:],
                        ps[:],
                        sc_sb[:, sci : sci + 1],
                        bias_sb[:, o : o + 1],
                        mybir.AluOpType.mult,
                        mybir.AluOpType.add,
                    )
                nc.scalar.dma_start(yt[o * P : (o + 1) * P, t0 : t0 + tsz], ot[:])

            # Group 0 (o-blocks 0..GRP-1) runs k-major so every arriving
            # X-slab immediately feeds GRP o-columns of PE work; it owns all
            # 2*GRP PSUM banks.
            g0 = list(range(GRP))
            wovs0 = [
                w_pref.pop(o)[:]
                .bitcast(bf16)
                .rearrange("p k (o two) -> p k o two", two=2)
                for o in g0
            ]
            pss0 = {
                (oi, ti): psump.tile(
                    [P, tsz], f32, tag=f"ps{oi}_{ti}", name=f"ps{oi}_{ti}"
                )
                for oi in g0
                for ti, (t0, tsz) in enumerate(chunks)
            }
            for k in range(kb):
                xvk = xqs[k // XQ]
                for oi in g0:
                    for ti, (t0, tsz) in enumerate(chunks):
                        nc.tensor.matmul(
                            pss0[oi, ti][:],
                            wovs0[oi][:, k, :, 1],
                            xvk[:, k % XQ, t0 : t0 + tsz, 1],
                            start=(k == 0),
                            stop=(k == kb - 1),
                        )
            for oi in g0:
                for ti, (t0, tsz) in enumerate(chunks):
                    evac(pss0[oi, ti], oi, t0, tsz, (oi + ti) % 2)

            # Remaining o-blocks run one at a time: per-bank k-runs rotate
            # through the PSUM banks (released by group 0 in the same
            # order), and evacuations pipeline under the next bank's MMs.
            for o in range(GRP, ob):
                w_o = w_pref.pop(o) if o in w_pref else load_w(o)
                wb = wbp.tile([P, kb, P], bf16, tag="wb", name=f"wb{o}")
                nc.vector.tensor_copy(wb[:], w_o[:])
                for ti, (t0, tsz) in enumerate(chunks):
                    ps = psump.tile(
                        [P, tsz],
                        f32,
                        tag=f"ps{o % GRP}_{ti}",
                        name=f"ps{o}_{ti}",
                    )
                    for k in range(kb):
                        nc.tensor.matmul(
                            ps[:],
                            wb[:, k, :],
                            xqs[k // XQ][:, k % XQ, t0 : t0 + tsz, 1],
                            start=(k == 0),
                            stop=(k == kb - 1),
                        )
                    evac(ps, o, t0, tsz, (o + ti) % 2, sci=1)
    nc.finalize()
    return nc


def _trunc_ratio(a: np.ndarray) -> float:
    """mean(|trunc_bf16(a)|) / mean(|a|) — the systematic shrink from
    reading only the high 16 bits of each fp32."""
    t = (a.view(np.uint32) & np.uint32(0xFFFF0000)).view(np.float32)
    denom = float(np.abs(a).sum())
    if denom == 0.0:
        return 1.0
    return float(np.abs(t).sum()) / denom


def _prepare(inputs, weight, bias, group_sizes):
    """Build (or reuse) the program and the per-core input maps."""
    inputs = np.ascontiguousarray(np.asarray(inputs, dtype=np.float32))
    weight = np.ascontiguousarray(np.asarray(weight, dtype=np.float32))
    bias = np.ascontiguousarray(np.asarray(bias, dtype=np.float32))
    g = np.asarray(group_sizes).astype(np.int64)

    t_tokens, n_in = inputs.shape
    n_exp, _, n_out = weight.shape
    assert n_exp == N_CORES, f"expected {N_CORES} experts, got {n_exp}"
    offs = np.concatenate([[0], np.cumsum(g)])
    assert offs[-1] == t_tokens, "group_sizes must sum to token count"

    maxg = max(P, int(-(-int(g.max()) // P)) * P)

    key = (maxg, n_in, n_out)
    if key not in _BUILD_CACHE:
        _BUILD_CACHE[key] = _build_program(maxg, n_in, n_out)
    nc = _BUILD_CACHE[key]

    ob = n_out // P
    bias_host = np.ascontiguousarray(bias.reshape(ob, P).T)  # [P, ob]

    # Compensate the mean truncation shrink: col 0 for truncated X and W
    # (group 0), col 1 for truncated X with round-to-nearest W (later os).
    rx, rw = _trunc_ratio(inputs), _trunc_ratio(weight)
    sc_host = np.empty((P, 2), np.float32)
    sc_host[:, 0] = 1.0 / (rx * rw)
    sc_host[:, 1] = 1.0 / rx

    in_maps = []
    for e in range(n_exp):
        xe = inputs[offs[e] : offs[e + 1]]  # [g_e, n_in]
        xt_e = np.zeros((n_in, maxg), np.float32)
        xt_e[:, : g[e]] = xe.T
        w_e = np.ascontiguousarray(
            weight[e].reshape(kb := n_in // P, P, ob, P).transpose(2, 1, 0, 3)
        )  # [ob, P(k within block), kb, P(o)]
        in_maps.append(
            {"xt": xt_e, "w": w_e, "bias": bias_host, "sc": sc_host}
        )
    return nc, in_maps, g, offs, (t_tokens, n_out)


def kernel(inputs, weight, bias, group_sizes):
    nc, in_maps, g, offs, (t_tokens, n_out) = _prepare(
        inputs, weight, bias, group_sizes
    )
    res = run_bass_kernel_spmd(nc, in_maps, core_ids=list(range(N_CORES)))

    out = np.empty((t_tokens, n_out), np.float32)
    for e in range(N_CORES):
        if g[e] == 0:
            continue
        yt_e = res.results[e]["yt"]  # [n_out, maxg]
        out[offs[e] : offs[e + 1]] = yt_e[:, : g[e]].T
    return out



# revision 4
# speedup vs baseline: 1.4127x; 1.4127x over previous
"""MoE grouped linear (gmm) kernel for 8 Trainium2 NeuronCores.

Strategy (balanced 2D split: 2 K-halves x 4 out-quarters):
  - The expert-parallel baseline pads every core to the max group size, so
    the slowest core does up to maxg/mean extra PE work. Instead, EVERY
    core processes ALL T tokens over (IN/2) contraction rows and (OUT/4)
    output columns: exactly T*IN*OUT/8 MACs per core regardless of the
    group-size skew, and the instruction stream is identical across cores
    (group boundaries are the same for everyone) - clean SPMD.
  - Core c = (kh, oq) gets X^T[kh-half] and, for all E experts,
    W_e[kh-half, oq-quarter]. Weight HBM traffic is exactly one
    expert-equivalent per core (no replication); X is replicated 4x but
    only half-length. Everything streams as fp16 (host-converted,
    round-to-nearest), accumulates in fp32 PSUM.
  - Tokens stay sorted by expert; matmul token chunks split at the group
    boundaries (baked into the program from the actual group_sizes).
  - Each core writes its fp16 partial [OUT/4, T]; host sums the two
    K-half partials in fp32, adds the bias, and reassembles [T, OUT].
  - DMA (in: 8.4M W + 8.4M X, out: 4.2M partials per core) and PE
    (2.15G MACs ~ 55us) are both near the roofline ridge; input DMA is
    issued in compute-consumption order so the PE ramps up early.
"""

import numpy as np

import concourse.bass as bass
from concourse import bacc
import concourse.mybir as mybir
import concourse.tile as tile
from concourse.bass_utils import run_bass_kernel_spmd

N_CORES = 8
KSPLIT = 2   # contraction-dim split factor
OSPLIT = 4   # output-dim split factor
P = 128
CHUNK = 512  # token-chunk (PSUM bank = 512 fp32)

_BUILD_CACHE: dict = {}


def _chunk_segments(groups, t_tokens):
    """Per 512-token chunk: (t0, clen, [(off_in_chunk, len, expert)])."""
    bounds = np.concatenate([[0], np.cumsum(np.asarray(groups, np.int64))])
    out = []
    t0 = 0
    while t0 < t_tokens:
        cl = min(CHUNK, t_tokens - t0)
        segs = []
        for e in range(len(groups)):
            s = max(t0, int(bounds[e]))
            t = min(t0 + cl, int(bounds[e + 1]))
            if t > s:
                segs.append((s - t0, t - s, e))
        out.append((t0, cl, segs))
        t0 += cl
    return out


def _build_program(t_tokens, n_in, n_out, n_exp, groups):
    kh = n_in // KSPLIT    # contraction rows per core
    kb = kh // P           # k-blocks
    oq = n_out // OSPLIT   # out cols per core
    ob = oq // P           # o-blocks
    chunks = _chunk_segments(groups, t_tokens)
    nch = len(chunks)
    tp = nch * CHUNK       # padded token count for the X layout
    f16 = mybir.dt.float16
    f32 = mybir.dt.float32
    eh = (n_exp + 1) // 2  # experts per W DMA piece

    nc = bacc.Bacc(
        "TRN2", target_bir_lowering=False, debug=False, num_devices=N_CORES
    )
    x = nc.dram_tensor("x", [nch, P, kb, CHUNK], f16, kind="ExternalInput")
    w = nc.dram_tensor("w", [P, ob, n_exp, kb, P], f16, kind="ExternalInput")
    y = nc.dram_tensor("y", [ob, P, t_tokens], f16, kind="ExternalOutput")

    # Unit (o-block, chunk) order co-designed with the input DMA order so
    # the PE starts early and never waits long: W arrives per o-block,
    # X per chunk; early units use the o-blocks/chunks already resident.
    units = []
    if ob == 4 and nch == 8:
        for o_, c_ in [(0, 0), (0, 1), (1, 0), (1, 1), (0, 2), (1, 2),
                       (2, 0), (2, 1), (2, 2), (0, 3), (1, 3), (2, 3),
                       (3, 0), (3, 1), (3, 2), (3, 3)]:
            units.append((o_, c_))
        for c_ in range(4, 8):
            for o_ in range(4):
                units.append((o_, c_))
    else:  # generic fallback: diagonal frontier
        for s in range(ob + nch - 1):
            for o_ in range(ob):
                if 0 <= s - o_ < nch:
                    units.append((o_, s - o_))

    with tile.TileContext(nc) as tc:
        with (
            tc.tile_pool(name="const", bufs=1) as constp,
            tc.tile_pool(name="xsb", bufs=1) as xp,
            tc.tile_pool(name="wsb", bufs=1) as wp,
            tc.tile_pool(name="outsb", bufs=4) as outp,
            tc.tile_pool(name="psum", bufs=1, space="PSUM") as psump,
        ):
            x_t = {}
            w_t = {}

            def load_x(c_):
                x_t[c_] = xp.tile([P, kb, CHUNK], f16, tag=f"x{c_}",
                                  name=f"x{c_}")
                nc.sync.dma_start(x_t[c_][:], x[c_])

            def load_w(o_, h_):
                e0 = h_ * eh
                e1 = min(n_exp, e0 + eh)
                w_t[(o_, h_)] = wp.tile([P, e1 - e0, kb, P], f16,
                                        tag=f"w{o_}_{h_}", name=f"w{o_}_{h_}")
                nc.sync.dma_start(w_t[(o_, h_)][:], w[:, o_, e0:e1])

            # Input DMA order (one HW ring -> delivered in this order),
            # matched to the unit order above.
            nhalf = 2 if n_exp > eh else 1
            if ob == 4 and nch == 8:
                dma_seq = ["x0", "w0", "x1", "w1", "x2", "w2", "x3", "w3",
                           "x4", "x5", "x6", "x7"]
            else:
                dma_seq = []
                for i in range(max(ob, nch)):
                    if i < nch:
                        dma_seq.append(f"x{i}")
                    if i < ob:
                        dma_seq.append(f"w{i}")
            for item in dma_seq:
                if item[0] == "x":
                    load_x(int(item[1:]))
                else:
                    for h_ in range(nhalf):
                        load_w(int(item[1:]), h_)

            # HAM warmup: dependency-free matmuls issue while the prologue
            # DMAs stream so the PE clock is at 2.4 GHz for the real work.
            warm = constp.tile([P, 512], f16)
            nc.gpsimd.memset(warm[:], 0)
            ps_warm = psump.tile([P, CHUNK], f32, tag="b0", name="warmps")
            for i in range(24):
                nc.tensor.matmul(
                    ps_warm[:], warm[:, :P], warm[:],
                    start=(i == 0), stop=(i == 23),
                )

            for ui, (o_, c_) in enumerate(units):
                t0, cl, segs = chunks[c_]
                ps = psump.tile([P, cl], f32, tag=f"b{ui % 8}",
                                name=f"ps{o_}_{c_}")
                for (off, ln, e) in segs:
                    wt = w_t[(o_, e // eh)]
                    for k in range(kb):
                        nc.tensor.matmul(
                            ps[:, off:off + ln],
                            wt[:, e % eh, k, :],
                            x_t[c_][:, k, off:off + ln],
                            start=(k == 0), stop=(k == kb - 1),
                        )
                ot = outp.tile([P, cl], f16, tag="ot", name=f"ot{o_}_{c_}")
                if ui % 2 == 0:
                    nc.scalar.activation(
                        ot[:], ps[:], mybir.ActivationFunctionType.Identity
                    )
                else:
                    nc.vector.tensor_copy(ot[:], ps[:])
                nc.scalar.dma_start(y[o_, :, t0:t0 + cl], ot[:])
    nc.finalize()
    return nc


def _prepare(inputs, weight, bias, group_sizes):
    """Build (or reuse) the program and the per-core input maps."""
    inputs = np.asarray(inputs, dtype=np.float32)
    weight = np.asarray(weight, dtype=np.float32)
    bias = np.asarray(bias, dtype=np.float32)
    g = np.asarray(group_sizes).astype(np.int64)

    t_tokens, n_in = inputs.shape
    n_exp, _, n_out = weight.shape
    assert n_in % (KSPLIT * P) == 0 and n_out % (OSPLIT * P) == 0
    assert int(g.sum()) == t_tokens, "group_sizes must sum to token count"

    key = (t_tokens, n_in, n_out, n_exp, tuple(int(v) for v in g))
    if key not in _BUILD_CACHE:
        _BUILD_CACHE[key] = _build_program(
            t_tokens, n_in, n_out, n_exp, tuple(int(v) for v in g)
        )
    nc = _BUILD_CACHE[key]

    kh = n_in // KSPLIT
    kb = kh // P
    oq = n_out // OSPLIT
    ob = oq // P
    nch = (t_tokens + CHUNK - 1) // CHUNK
    tp = nch * CHUNK

    # X^T in fp16, chunk-major: [nch, P, kb, CHUNK] per K-half.
    xt = np.zeros((n_in, tp), np.float16)
    xt[:, :t_tokens] = inputs.T.astype(np.float16)
    x_halves = []
    for khi in range(KSPLIT):
        sl = xt[khi * kh:(khi + 1) * kh]                  # [kh, tp]
        sl = sl.reshape(kb, P, nch, CHUNK).transpose(2, 1, 0, 3)
        x_halves.append(np.ascontiguousarray(sl))         # [nch, P, kb, CHUNK]

    w16 = weight.astype(np.float16)                       # [E, n_in, n_out]
    in_maps = []
    for c in range(N_CORES):
        khi, oqi = c // OSPLIT, c % OSPLIT
        wsl = w16[:, khi * kh:(khi + 1) * kh, oqi * oq:(oqi + 1) * oq]
        # [E, kh, oq] -> [P(k in block), ob, E, kb, P(o)]
        wsl = wsl.reshape(n_exp, kb, P, ob, P).transpose(2, 3, 0, 1, 4)
        in_maps.append({
            "x": x_halves[khi],
            "w": np.ascontiguousarray(wsl),
        })
    return nc, in_maps, g, (t_tokens, n_out), (ob, oq)


def kernel(inputs, weight, bias, group_sizes):
    nc, in_maps, g, (t_tokens, n_out), (ob, oq) = _prepare(
        inputs, weight, bias, group_sizes
    )
    res = run_bass_kernel_spmd(nc, in_maps, core_ids=list(range(N_CORES)))

    bias = np.asarray(bias, dtype=np.float32)
    out = np.empty((t_tokens, n_out), np.float32)
    for oqi in range(OSPLIT):
        acc = res.results[oqi]["y"].astype(np.float32)
        for khi in range(1, KSPLIT):
            acc += res.results[khi * OSPLIT + oqi]["y"].astype(np.float32)
        out[:, oqi * oq:(oqi + 1) * oq] = acc.reshape(oq, t_tokens).T
    out += bias[None, :]
    return out


# revision 6
# speedup vs baseline: 1.4286x; 1.0113x over previous
"""MoE grouped linear (gmm) kernel for 8 Trainium2 NeuronCores.

Strategy (balanced 2D split: 2 K-halves x 4 out-quarters):
  - The expert-parallel baseline pads every core to the max group size, so
    the slowest core does up to maxg/mean extra PE work. Instead, EVERY
    core processes ALL T tokens over (IN/2) contraction rows and (OUT/4)
    output columns: exactly T*IN*OUT/8 MACs per core regardless of the
    group-size skew, and the instruction stream is identical across cores
    (group boundaries are the same for everyone) - clean SPMD.
  - Core c = (kh, oq) gets X^T[kh-half] and, for all E experts,
    W_e[kh-half, oq-quarter]. Weight HBM traffic is exactly one
    expert-equivalent per core (no replication); X is replicated 4x but
    only half-length. Everything streams as fp16 (host-converted,
    round-to-nearest), accumulates in fp32 PSUM.
  - Tokens stay sorted by expert; matmul token chunks split at the group
    boundaries (baked into the program from the actual group_sizes).
  - Each core writes its fp16 partial [OUT/4, T]; host sums the two
    K-half partials in fp32, adds the bias, and reassembles [T, OUT].
  - Input DMA is issued in compute-consumption order (chunk-major units,
    W sliced per (o-block, expert-pair)) so the PE starts as soon as the
    first ~1 MB lands and then never starves; a short HAM-warmup matmul
    burst on the last PSUM bank keeps the PE clock at 2.4 GHz without
    blocking the first real unit. Outputs gather 4 o-blocks per token
    chunk into one DMA to keep the semaphore count (and the end-of-
    program per-semaphore reset tail) small.
"""

import numpy as np

import concourse.bass as bass
from concourse import bacc
import concourse.mybir as mybir
import concourse.tile as tile
from concourse.bass_utils import run_bass_kernel_spmd

N_CORES = 8
KSPLIT = 2   # contraction-dim split factor
OSPLIT = 4   # output-dim split factor
P = 128
CHUNK = 512  # token-chunk (PSUM bank = 512 fp32)

_BUILD_CACHE: dict = {}


def _chunk_segments(groups, t_tokens):
    """Per 512-token chunk: (t0, clen, [(off_in_chunk, len, expert)])."""
    bounds = np.concatenate([[0], np.cumsum(np.asarray(groups, np.int64))])
    out = []
    t0 = 0
    while t0 < t_tokens:
        cl = min(CHUNK, t_tokens - t0)
        segs = []
        for e in range(len(groups)):
            s = max(t0, int(bounds[e]))
            t = min(t0 + cl, int(bounds[e + 1]))
            if t > s:
                segs.append((s - t0, t - s, e))
        out.append((t0, cl, segs))
        t0 += cl
    return out


def _build_program(t_tokens, n_in, n_out, n_exp, groups):
    kh = n_in // KSPLIT    # contraction rows per core
    kb = kh // P           # k-blocks
    oq = n_out // OSPLIT   # out cols per core
    ob = oq // P           # o-blocks
    chunks = _chunk_segments(groups, t_tokens)
    nch = len(chunks)
    f16 = mybir.dt.float16
    f32 = mybir.dt.float32

    # W DMA pieces: (o-block, expert-pair). npair pairs cover all experts.
    npair = (n_exp + 1) // 2

    nc = bacc.Bacc(
        "TRN2", target_bir_lowering=False, debug=False, num_devices=N_CORES
    )
    x = nc.dram_tensor("x", [nch, P, kb, CHUNK], f16, kind="ExternalInput")
    w = nc.dram_tensor("w", [P, ob, n_exp, kb, P], f16, kind="ExternalInput")
    y = nc.dram_tensor("y", [P, ob, t_tokens], f16, kind="ExternalOutput")

    # Input DMA order: x-chunk pieces and W (ob, expert-pair) pieces
    # interleaved so each chunk's weights land just before its x does.
    # x0 is split at its first segment boundary so the very first matmul
    # only waits for ~1 MB of DMA.
    pieces = []
    emitted_pairs = set()
    for c, (t0, cl, segs) in enumerate(chunks):
        need = [pr for pr in dict.fromkeys(e // 2 for (_o, _l, e) in segs)
                if pr not in emitted_pairs]
        emitted_pairs.update(need)
        if c == 0:
            xs = segs[0][1] if len(segs) > 1 else cl
            pieces.append(("x", 0, 0, xs))
            for pr in need:
                pieces.append(("w", 0, pr, None))
            if xs < cl:
                pieces.append(("x", 0, xs, cl))
            for oi in range(1, ob):
                for pr in need:
                    pieces.append(("w", oi, pr, None))
        else:
            for oi in range(ob):
                for pr in need:
                    pieces.append(("w", oi, pr, None))
            pieces.append(("x", c, 0, cl))

    with tile.TileContext(nc) as tc:
        with (
            tc.tile_pool(name="const", bufs=1) as constp,
            tc.tile_pool(name="xsb", bufs=1) as xp,
            tc.tile_pool(name="wsb", bufs=1) as wp,
            tc.tile_pool(name="outsb", bufs=3) as outp,
            tc.tile_pool(name="psum", bufs=1, space="PSUM") as psump,
        ):
            x_t = {}
            w_t = {}
            for kind, a, b, cend in pieces:
                if kind == "x":
                    if a not in x_t:
                        x_t[a] = xp.tile([P, kb, chunks[a][1]], f16,
                                         tag=f"x{a}", name=f"x{a}")
                    nc.sync.dma_start(
                        x_t[a][:, :, b:cend], x[a][:, :, b:cend]
                    )
                else:
                    e0 = b * 2
                    e1 = min(n_exp, e0 + 2)
                    w_t[(a, b)] = wp.tile([P, e1 - e0, kb, P], f16,
                                          tag=f"w{a}_{b}", name=f"w{a}_{b}")
                    nc.sync.dma_start(w_t[(a, b)][:], w[:, a, e0:e1])

            # HAM warmup: dependency-free matmuls while the prologue DMAs
            # stream, sized to end roughly when the first real data lands.
            # Uses the last PSUM bank so the first real units are not
            # blocked behind it.
            warm = constp.tile([P, CHUNK], f16)
            nc.gpsimd.memset(warm[:], 0)
            ps_warm = psump.tile([P, CHUNK], f32, tag="b7", name="warmps")
            for i in range(28):
                nc.tensor.matmul(
                    ps_warm[:], warm[:, :P], warm[:],
                    start=(i == 0), stop=(i == 27),
                )

            # Chunk-major units: all o-blocks of a token chunk back to
            # back, gathered into one output tile -> one DMA per chunk.
            ui = 0
            for c, (t0, cl, segs) in enumerate(chunks):
                ot = outp.tile([P, ob, cl], f16, tag="ot", name=f"ot{c}")
                for oi in range(ob):
                    ps = psump.tile([P, cl], f32, tag=f"b{ui % 8}",
                                    name=f"ps{oi}_{c}")
                    for (off, ln, e) in segs:
                        wt = w_t[(oi, e // 2)]
                        for k in range(kb):
                            nc.tensor.matmul(
                                ps[:, off:off + ln],
                                wt[:, e % 2, k, :],
                                x_t[c][:, k, off:off + ln],
                                start=(k == 0), stop=(k == kb - 1),
                            )
                    if (c + oi) % 2 == 0:
                        nc.scalar.activation(
                            ot[:, oi, :], ps[:],
                            mybir.ActivationFunctionType.Identity,
                        )
                    else:
                        nc.vector.tensor_copy(ot[:, oi, :], ps[:])
                    ui += 1
                nc.scalar.dma_start(y[:, :, t0:t0 + cl], ot[:])
    nc.finalize()
    return nc


def _prepare(inputs, weight, bias, group_sizes):
    """Build (or reuse) the program and the per-core input maps."""
    inputs = np.asarray(inputs, dtype=np.float32)
    weight = np.asarray(weight, dtype=np.float32)
    bias = np.asarray(bias, dtype=np.float32)
    g = np.asarray(group_sizes).astype(np.int64)

    t_tokens, n_in = inputs.shape
    n_exp, _, n_out = weight.shape
    assert n_in % (KSPLIT * P) == 0 and n_out % (OSPLIT * P) == 0
    assert int(g.sum()) == t_tokens, "group_sizes must sum to token count"

    key = (t_tokens, n_in, n_out, n_exp, tuple(int(v) for v in g))
    if key not in _BUILD_CACHE:
        _BUILD_CACHE[key] = _build_program(
            t_tokens, n_in, n_out, n_exp, tuple(int(v) for v in g)
        )
    nc = _BUILD_CACHE[key]

    kh = n_in // KSPLIT
    kb = kh // P
    oq = n_out // OSPLIT
    ob = oq // P
    nch = (t_tokens + CHUNK - 1) // CHUNK
    tp = nch * CHUNK

    # X^T in fp16, chunk-major: [nch, P, kb, CHUNK] per K-half.
    xt = np.zeros((n_in, tp), np.float16)
    xt[:, :t_tokens] = inputs.T.astype(np.float16)
    x_halves = []
    for khi in range(KSPLIT):
        sl = xt[khi * kh:(khi + 1) * kh]                  # [kh, tp]
        sl = sl.reshape(kb, P, nch, CHUNK).transpose(2, 1, 0, 3)
        x_halves.append(np.ascontiguousarray(sl))         # [nch, P, kb, CHUNK]

    w16 = weight.astype(np.float16)                       # [E, n_in, n_out]
    in_maps = []
    for c in range(N_CORES):
        khi, oqi = c // OSPLIT, c % OSPLIT
        wsl = w16[:, khi * kh:(khi + 1) * kh, oqi * oq:(oqi + 1) * oq]
        # [E, kh, oq] -> [P(k in block), ob, E, kb, P(o)]
        wsl = wsl.reshape(n_exp, kb, P, ob, P).transpose(2, 3, 0, 1, 4)
        in_maps.append({
            "x": x_halves[khi],
            "w": np.ascontiguousarray(wsl),
        })
    return nc, in_maps, g, (t_tokens, n_out), (ob, oq)


def kernel(inputs, weight, bias, group_sizes):
    nc, in_maps, g, (t_tokens, n_out), (ob, oq) = _prepare(
        inputs, weight, bias, group_sizes
    )
    res = run_bass_kernel_spmd(nc, in_maps, core_ids=list(range(N_CORES)))

    bias = np.asarray(bias, dtype=np.float32)
    out = np.empty((t_tokens, n_out), np.float32)
    for oqi in range(OSPLIT):
        acc = res.results[oqi]["y"].astype(np.float32)
        for khi in range(1, KSPLIT):
            acc += res.results[khi * OSPLIT + oqi]["y"].astype(np.float32)
        # y is [P, ob, T]; out column within the quarter = ob*P + p
        out[:, oqi * oq:(oqi + 1) * oq] = (
            acc.transpose(1, 0, 2).reshape(oq, t_tokens).T
        )
    out += bias[None, :]
    return out
